# revision 11
# baseline (speedup 1.0000x reference)
"""Trainium2 Bass kernel for DiffVorticeSketchRender.

Sharding: 8 cores = 4 batches x 2 H-halves (64 rows each + 3-4 row halos).
Device layout: [D=128 partitions, H slices, W free] everywhere.
- curl + fdiffs: PSUM-accumulated matmuls with +/-I and D-difference band
  matrices (H/W shifts via shifted rhs access patterns, W edge handled by a
  host-extrapolated 129th column, D edge inside the band matrix).
- 3D gaussian smooth (separable): 7 accumulated matmuls fuse the D-conv
  (band matrix) with the H-conv (shifted slice windows), then 7 accumulated
  identity matmuls with shifted W windows for the W-conv.
- depth flip + cumsum: one suffix-sum triangular matmul.
- transmittance/integration: exp on ScalarE, band-matrix matmul for the
  trapezoid coefficients, ones/e127 reduction matmuls. All fp32r, N>=256.
"""

import numpy as np

import concourse.bacc as bacc
import concourse.bass as bass
import concourse.mybir as mybir
import concourse.tile as tile
from concourse.bass_utils import run_bass_kernel_spmd

F32 = mybir.dt.float32
F32R = mybir.dt.float32r
AL = mybir.AluOpType
AF = mybir.ActivationFunctionType

KHS, SIGMA, C = 3, 1.6, 20.0


def _gauss1d():
    size = 2 * KHS + 1
    g = np.arange(size, dtype=np.float64) - (size - 1) / 2.0
    g = np.exp(-((g / SIGMA) ** 2) / 2.0) / (SIGMA * np.sqrt(2.0 * np.pi))
    return (g / g.sum()).astype(np.float32)


GK = _gauss1d()


def _const_mats():
    mdz = np.zeros((128, 128), np.float32)
    for d in range(127):
        mdz[d, d] = -1.0
        mdz[d, d + 1] = 1.0
    mdz[127, 126] = -1.0
    mdz[127, 127] = 1.0

    bd = np.zeros((128, 128), np.float32)
    for dp in range(128):
        for k in range(7):
            d = dp + k - 3
            if 0 <= d < 128:
                bd[dp, d] = GK[k]

    mc = np.zeros((128, 128), np.float32)
    mc[0, 0], mc[0, 1] = -0.5, 0.5
    for k in range(1, 127):
        mc[k, k - 1], mc[k, k + 1] = -0.5, 0.5
    mc[127, 126], mc[127, 127] = -0.5, -0.5

    eye = np.eye(128, dtype=np.float32)
    kbd = np.stack([(GK[k] * bd).T for k in range(7)], axis=1)  # [128,7,128] lhsT, D+H pass
    ki = np.stack([GK[k] * eye for k in range(7)], axis=1)      # [128,7,128] lhsT, W pass
    suf = (np.arange(128)[:, None] >= np.arange(128)[None, :]).astype(np.float32)
    red = np.zeros((128, 2), np.float32)
    red[:, 0] = 1.0
    red[127, 1] = 1.0
    return {
        "KBD": kbd, "KI": ki, "CIP": eye, "CIN": -eye,
        "MDZT": mdz.T.copy(), "MDZTN": (-mdz.T).copy(),
        "SUF": suf, "MCT": mc.T.copy(), "RED": red,
    }


def _curl_groups():
    gs = []
    s0 = 0
    while s0 < 70:
        cnt = min(4, 70 - s0)
        gs.append((s0, cnt))
        s0 += cnt
    return gs


def build_program():
    nc = bacc.Bacc("TRN2", target_bir_lowering=False, debug=False)

    d_in = nc.dram_tensor("d_in", [128, 70, 128], F32R, kind="ExternalInput")
    v_in = nc.dram_tensor("v_in", [3, 128, 71, 129], F32R, kind="ExternalInput")
    m0_in = nc.dram_tensor("m0_in", [128, 3, 128], F32, kind="ExternalInput")
    m1_in = nc.dram_tensor("m1_in", [128, 3, 128], F32, kind="ExternalInput")
    cm = _const_mats()
    c_in = {}
    for name, arr in cm.items():
        c_in[name] = nc.dram_tensor(f"c_{name}", list(arr.shape), F32R,
                                    kind="ExternalInput")
    zpad_in = nc.dram_tensor("zpad", [128, 64, 6], F32R, kind="ExternalInput")
    out_t = nc.dram_tensor("out", [1, 8192], F32, kind="ExternalOutput")

    with tile.TileContext(nc) as tc:
        with tc.tile_pool(name="const", bufs=1) as cpool, \
             tc.tile_pool(name="vols", bufs=1) as vol:
            ct = {}
            for name, arr in cm.items():
                t = cpool.tile(list(arr.shape), F32R, tag=f"c_{name}")
                nc.sync.dma_start(t[:], c_in[name][:])
                ct[name] = t
            m0t = cpool.tile([128, 3, 128], F32, tag="m0")
            m1t = cpool.tile([128, 3, 128], F32, tag="m1")
            nc.sync.dma_start(m0t[:], m0_in[:])
            nc.sync.dma_start(m1t[:], m1_in[:])

            vn = vol.tile([128, 70, 128], F32R, tag="vn")

            # ---- stage 1: curl + |curl|^2 (scoped so v frees after) ----
            with tc.tile_pool(name="vdata", bufs=1) as vp, \
                 tc.tile_pool(name="sq", bufs=3) as sqp, \
                 tc.tile_pool(name="cpsum", bufs=2,
                              space=bass.MemorySpace.PSUM) as cps:
                du = vp.tile([128, 71, 129], F32R, tag="du")
                dv = vp.tile([128, 71, 129], F32R, tag="dv")
                dw = vp.tile([128, 71, 129], F32R, tag="dw")
                # chunk channel loads so early curl groups overlap the DMA
                for a, b in ((0, 13), (13, 25), (25, 37), (37, 49),
                             (49, 61), (61, 71)):
                    nc.sync.dma_start(du[:, a:b, :], v_in[0, :, a:b, :])
                    nc.sync.dma_start(dv[:, a:b, :], v_in[1, :, a:b, :])
                    nc.sync.dma_start(dw[:, a:b, :], v_in[2, :, a:b, :])

                for (s0, cnt) in _curl_groups():
                    n = cnt * 128
                    pcu = cps.tile([128, cnt, 128], F32, tag="pcu")
                    pcv = cps.tile([128, cnt, 128], F32, tag="pcv")
                    pcw = cps.tile([128, cnt, 128], F32, tag="pcw")
                    nc.tensor.matmul(pcu[:], ct["CIP"][:],
                                     dw[:, s0 + 1:s0 + 1 + cnt, 0:128],
                                     start=True, stop=False)
                    nc.tensor.matmul(pcu[:], ct["CIN"][:],
                                     dw[:, s0:s0 + cnt, 0:128],
                                     start=False, stop=False)
                    nc.tensor.matmul(pcu[:], ct["MDZTN"][:],
                                     dv[:, s0:s0 + cnt, 0:128], start=False, stop=True)

                    nc.tensor.matmul(pcv[:], ct["MDZT"][:],
                                     du[:, s0:s0 + cnt, 0:128], start=True, stop=False)
                    nc.tensor.matmul(pcv[:], ct["CIN"][:],
                                     dw[:, s0:s0 + cnt, 1:129],
                                     start=False, stop=False)
                    nc.tensor.matmul(pcv[:], ct["CIP"][:],
                                     dw[:, s0:s0 + cnt, 0:128], start=False, stop=True)

                    nc.tensor.matmul(pcw[:], ct["CIP"][:],
                                     dv[:, s0:s0 + cnt, 1:129], start=True, stop=False)
                    nc.tensor.matmul(pcw[:], ct["CIN"][:],
                                     dv[:, s0:s0 + cnt, 0:128],
                                     start=False, stop=False)
                    nc.tensor.matmul(pcw[:], ct["CIN"][:],
                                     du[:, s0 + 1:s0 + 1 + cnt, 0:128],
                                     start=False, stop=False)
                    nc.tensor.matmul(pcw[:], ct["CIP"][:],
                                     du[:, s0:s0 + cnt, 0:128], start=False, stop=True)

                    squ = sqp.tile([128, cnt, 128], F32, tag="squ")
                    sqv = sqp.tile([128, cnt, 128], F32, tag="sqv")
                    sqw = sqp.tile([128, cnt, 128], F32, tag="sqw")
                    nc.scalar.activation(squ[:], pcu[:], AF.Square)
                    nc.scalar.activation(sqv[:], pcv[:], AF.Square)
                    nc.scalar.activation(sqw[:], pcw[:], AF.Square)
                    tsum = sqp.tile([128, cnt, 128], F32, tag="tsum")
                    nc.vector.tensor_add(tsum[:], squ[:], sqv[:])
                    nc.vector.tensor_add(vn[:, s0:s0 + cnt, :],
                                         tsum[:], sqw[:])

            # mask out-of-range boundary slices, then sqrt in place
            nc.vector.tensor_mul(vn[:, 0:3, :], vn[:, 0:3, :], m0t[:])
            nc.vector.tensor_mul(vn[:, 67:70, :], vn[:, 67:70, :], m1t[:])
            nc.scalar.activation(vn[:], vn[:], AF.Sqrt)

            # ---- stage 2/3: the two 3D smooths ----
            smp_cm = tc.tile_pool(name="smoothp", bufs=1)
            smp = smp_cm.__enter__()
            s1 = smp.tile([128, 64, 134], F32R, tag="s1")
            s1d = smp.tile([128, 64, 134], F32R, tag="s1d")
            for t in (s1, s1d):
                nc.sync.dma_start(t[:, :, 0:3], zpad_in[:, :, 0:3])
                nc.sync.dma_start(t[:, :, 131:134], zpad_in[:, :, 3:6])
            vns = smp.tile([128, 64, 128], F32R, tag="vns")
            dd = smp.tile([128, 70, 128], F32R, tag="dd")
            nc.sync.dma_start(dd[:], d_in[:])
            ds = smp.tile([128, 64, 128], F32R, tag="dd")

            def smooth(src, dst, s1):
                with tc.tile_pool(name="spsum", bufs=2,
                                  space=bass.MemorySpace.PSUM) as sps:
                    for go in range(16):
                        g4 = go * 4
                        p1 = sps.tile([128, 4, 128], F32, tag="p1")
                        for k in range(7):
                            nc.tensor.matmul(p1[:], ct["KBD"][:, k, :],
                                             src[:, g4 + k:g4 + k + 4, :],
                                             start=(k == 0), stop=(k == 6))
                        if go % 2 == 0:
                            nc.scalar.copy(s1[:, g4:g4 + 4, 3:131], p1[:])
                        else:
                            nc.vector.tensor_copy(s1[:, g4:g4 + 4, 3:131],
                                                  p1[:])
                    for go in range(16):
                        g4 = go * 4
                        p2 = sps.tile([128, 4, 128], F32, tag="p2")
                        for k in range(7):
                            nc.tensor.matmul(p2[:], ct["KI"][:, k, :],
                                             s1[:, g4:g4 + 4, k:k + 128],
                                             start=(k == 0), stop=(k == 6))
                        if go % 2 == 0:
                            nc.vector.tensor_copy(dst[:, g4:g4 + 4, :], p2[:])
                        else:
                            nc.scalar.copy(dst[:, g4:g4 + 4, :], p2[:])

            smooth(vn, vns, s1)
            smooth(dd, ds, s1d)

            # ---- stage 4: transmittance + trapezoid integration ----
            ivsb = smp.tile([1, 8192], F32, tag="s1")
            with tc.tile_pool(name="post", bufs=3) as pp, \
                 tc.tile_pool(name="ppsum", bufs=2,
                              space=bass.MemorySpace.PSUM) as pps:
                for cc in range(16):
                    g4 = cc * 4
                    ps = pps.tile([128, 4, 128], F32, tag="ps")
                    nc.tensor.matmul(ps[:], ct["SUF"][:], ds[:, g4:g4 + 4, :],
                                     start=True, stop=True)
                    ec = pp.tile([128, 4, 128], F32R, tag="ec")
                    bc = pp.tile([128, 4, 128], F32R, tag="bc")
                    nc.scalar.activation(ec[:], ps[:], AF.Exp, scale=-C)
                    nc.scalar.activation(bc[:], ps[:], AF.Copy, bias=1.0,
                                         scale=C)
                    nc.vector.tensor_mul(bc[:], bc[:], ec[:])
                    pc2 = pps.tile([128, 4, 128], F32, tag="pc2")
                    nc.tensor.matmul(pc2[:], ct["MCT"][:], bc[:],
                                     start=True, stop=True)
                    pchunk = pp.tile([128, 4, 128], F32R, tag="pchunk")
                    nc.vector.tensor_mul(pchunk[:], pc2[:],
                                         vns[:, g4:g4 + 4, :])
                    piv = pps.tile([1, 512], F32, tag="piv")
                    nc.tensor.matmul(piv[:], ct["RED"][:, 0:1], pchunk[:],
                                     start=True, stop=False)
                    nc.tensor.matmul(piv[:], ct["RED"][:, 1:2],
                                     vns[:, g4:g4 + 4, :], start=False, stop=True)
                    nc.vector.tensor_scalar_min(
                        ivsb[0:1, cc * 512:(cc + 1) * 512], piv[:], 1.0)
                nc.vector.tensor_scalar_max(ivsb[:], ivsb[:], 0.0)
                nc.sync.dma_start(out_t[:], ivsb[:])
            smp_cm.__exit__(None, None, None)

    nc.compile()
    return nc


def host_prepare(d_np, v_np):
    cores = []
    zeros3 = np.zeros((128, 3, 128), np.float32)
    ones3 = np.ones((128, 3, 128), np.float32)
    vext = np.zeros((3, 128, 135, 129), np.float32)
    cm = _const_mats()
    for c in range(8):
        b, hh = c // 2, c % 2
        h0 = 64 * hh
        dpad = np.zeros((128, 70, 128), np.float32)
        lo, hi = h0 - 3, h0 + 67
        src_lo, src_hi = max(lo, 0), min(hi, 128)
        dpad[:, (src_lo - lo):(src_hi - lo), :] = \
            d_np[b, 0, :, src_lo:src_hi, :]
        vext[:] = 0.0
        vext[:, :, 3:131, 0:128] = v_np[b]
        vext[:, :, 131, 0:128] = 2 * v_np[b, :, :, 127, :] - v_np[b, :, :, 126, :]
        vext[:, :, :, 128] = 2 * vext[:, :, :, 127] - vext[:, :, :, 126]
        vin = np.ascontiguousarray(vext[:, :, h0:h0 + 71, :])
        m = {
            "d_in": dpad, "v_in": vin,
            "zpad": np.zeros((128, 64, 6), np.float32),
            "m0_in": zeros3 if hh == 0 else ones3,
            "m1_in": zeros3 if hh == 1 else ones3,
        }
        for name, arr in cm.items():
            m[f"c_{name}"] = arr
        cores.append(m)
    return cores


_NC = None


def kernel(d, v):
    global _NC
    d = np.asarray(d, np.float32)
    v = np.asarray(v, np.float32)
    if _NC is None:
        _NC = build_program()
    in_maps = host_prepare(d, v)
    res = run_bass_kernel_spmd(_NC, in_maps, list(range(8)))
    out = np.zeros((4, 1, 128, 128), np.float32)
    for c in range(8):
        b, hh = c // 2, c % 2
        out[b, 0, 64 * hh:64 * hh + 64, :] = \
            res.results[c]["out"].reshape(64, 128)
    return out
